# revision 65
# baseline (speedup 1.0000x reference)
"""Trainium2 Bass kernel for nn_BetaMPERLGraphConvLayer (relational GNN layer).

Computation (see the problem's reference):
  per relation r: mean-aggregate neighbor features over edges
  (segment-sum by destination + degree normalize), concat the R supports,
  two basis-decomposed linear heads, relu+bias, 1.01+softplus.

Strategy:
  - Destination nodes are packed into 128-node tiles and the tiles are dealt
    across the 8 NeuronCores (host-side balanced packing -> one SPMD
    program).
  - Mixed-relation chunk packing: per (tile, half) a core's edges are sorted
    by relation and chunked every 128 with no per-relation alignment; every
    (chunk, relation) pair present on any core is a work item with its own
    one-hot column (so relation boundaries can straddle chunks).
  - Per 128-edge chunk the kernel gathers the 128 source rows with
    dma_gather (int16 indices -> X split into two <=32768-row half tables),
    builds one-hot [edge, dest-slot] columns on the vector engine
    (iota == dest), and scatter-adds via TensorE:
    psum[dest, feat] += onehot.T @ G_hi (bf16-hi only; rel err ~2e-3).
  - SWDGE descriptor generation is the hard bottleneck; it is spread over
    4 SWDGE queues (num_swdge_queues=4, round-robin queue_num) which
    parallelizes the Q7 desc-gen ucode (~100 desc/us per active queue,
    additive across queues).
  - Inverse degrees 1/(deg+eps) are exact, computed host-side from `rows`
    and shipped as a per-core input table (no degree matmuls on device).
  - Per-tile epilogue is split: normalize out of PSUM inline (ACT engine,
    frees the bank), then 3 batches later the PE transposes + two 512->64
    head matmuls + relu/bias/softplus/+1.01 run with inputs long ready
    (no PE pipeline bubble at tile close).

Measured (8 cores, full problem): 1.269 ms HW exec, rel err ~2e-3
(gate is 2e-2).  History: baseline 3.83 ms (single SWDGE queue, hi+lo
matmuls, device degrees); 1.92 ms after 4 SWDGE queues; 1.82 ms after
host inv-degree + hi-only matmul; 1.75 ms after mixed-relation packing
(466k -> 410k gather descriptors/core); 1.43 ms after JJ=16 -> 8;
1.287 ms after GSPLIT=2 (each batch's gather split into two 512-idx
dma_gather calls on different queues: 512-idx ucode quanta keep all 4
SWDGE queues fed -> sustained ~344 desc/us vs ~250 before); 1.269 ms
after rotating which queue pair consecutive batches co-start on
(extra qrr bump per batch) instead of pinning pairs {0,1}/{2,3}.
Known walls: desc-gen sustains ~344/430 desc/us (per-queue ~100/us,
additive); stream head ~36us + tail ~45us; DVE one-hot ~0.65 ms pure.
Measured SLOWER: JJ=32/16/6/4 variants, GSPLIT=4 (256-idx quanta too
fine, 1.69ms), JJ=16+GSPLIT=4 (same quanta, bigger batches, 1.47ms),
io bufs 12/14, 64KB dma scratch, (lo,hi)-total packing objective,
fp8e4 one-hot (correct but 1.31ms); single_packet=True crashes.
"""

import os
import sys
import time

for _p in ("/opt/trn_rl_repo", "/root/.axon_site/_ro/trn_rl_repo"):
    if os.path.isdir(_p) and _p not in sys.path:
        sys.path.insert(0, _p)

import numpy as np

# ---------------------------------------------------------------- constants
N_NODES = 50000
DIN = 64
DOUT = 64
R_REL = 8
B_BASES = 4
N_CORES = 8
P = 128
EPS = 1e-8
SHIFT = 1.01

SPLIT = 32767          # lo table: rows [0, 32767) + zero row at 32767
NT = 50                # dest tiles per core (50*128*8 = 51200 slots >= 50000)
JJ = 8                 # 128-edge chunks per gather batch
GSPLIT = 2             # gather instructions per batch (round-robin queues)
PAD_DST = 255.0        # one-hot target that never matches iota 0..127

_cache = {}


# ---------------------------------------------------------------- host prep
def _build_schedule(rows, cols):
    """Assign nodes to (core, tile, slot); build per-core edge chunk grids and
    the shared compile-time chunk schedule."""
    t0 = time.time()
    R, E = rows.shape
    TILES = N_CORES * NT

    half = (cols >= SPLIT).astype(np.int64)            # [R, E]
    # per-node degree split by (relation, half): [N, R*2]
    deg = np.zeros((N_NODES, R * 2), np.int64)
    for r in range(R):
        key = rows[r] * 2 + half[r]
        cnt = np.bincount(key, minlength=N_NODES * 2)
        deg[:, 2 * r] = cnt[0::2]
        deg[:, 2 * r + 1] = cnt[1::2]

    # exact per-(relation, node) inverse degree (device ships this as input)
    inv_node = 1.0 / ((deg[:, 0::2] + deg[:, 1::2]).astype(np.float64) + EPS)
    inv_node = inv_node.astype(np.float32)             # [N, R]

    # greedy vector bin-packing: nodes (desc by max group count) -> tiles
    order = np.argsort(-deg.max(1), kind="stable")
    counts = np.zeros((TILES, R * 2), np.int64)
    fill = np.zeros(TILES, np.int64)
    tile_of = np.empty(N_NODES, np.int32)
    slot_of = np.empty(N_NODES, np.int32)
    BIG = 1 << 40
    for n in order:
        d = deg[n]
        cand = (counts + d).max(1)
        cand[fill >= P] = BIG
        t = int(np.argmin(cand))
        tile_of[n] = t
        slot_of[n] = fill[t]
        counts[t] += d
        fill[t] += 1

    # deal tiles to cores: sort by total desc, tile i -> (core i%8, slot i//8)
    tord = np.argsort(-counts.sum(1), kind="stable")
    core_of_tile = np.empty(TILES, np.int32)
    slotT_of_tile = np.empty(TILES, np.int32)
    core_of_tile[tord] = np.arange(TILES) % N_CORES
    slotT_of_tile[tord] = np.arange(TILES) // N_CORES

    core_of = core_of_tile[tile_of]          # [N]
    tslot_of = slotT_of_tile[tile_of]        # [N] tile index within core
    # per (core, tslot, r, half) counts
    cnt4 = np.zeros((N_CORES, NT, R, 2), np.int64)
    for r in range(R):
        key = ((core_of[rows[r]] * NT + tslot_of[rows[r]]) * 2 + half[r])
        c = np.bincount(key, minlength=N_CORES * NT * 2)
        cnt4[:, :, r, :] = c.reshape(N_CORES, NT, 2)

    # mixed-relation chunk packing ------------------------------------------
    # Per (tslot, half): a core's edges are laid out sorted by relation and
    # chunked every 128 with no per-relation alignment; chunk count is the max
    # over cores.  Every (chunk, relation) pair present on ANY core becomes a
    # work item with its own one-hot column (relation boundaries straddle
    # chunks, so a chunk can carry 1-3 items).
    tot_th = cnt4.sum(2)                     # [m, NT, 2]
    Kth = (-(-tot_th // P)).max(0)           # [NT, 2] ceil-div, max over cores
    maxK = int(Kth.max())

    # rstart[m, t, h, r]: offset of relation r inside core m's (t,h) stream
    rstart = np.zeros((N_CORES, NT, 2, R + 1), np.int64)
    rstart[:, :, :, 1:] = np.cumsum(cnt4.transpose(0, 1, 3, 2), axis=3)

    present = np.zeros((NT, 2, maxK, R), bool)
    for m in range(N_CORES):
        for h in (0, 1):
            s = rstart[m, :, h, :-1]
            e = rstart[m, :, h, 1:]
            for t in range(NT):
                for r in range(R):
                    if e[t, r] > s[t, r]:
                        present[t, h, s[t, r] // P:(e[t, r] - 1) // P + 1,
                                r] = True

    # chunk streams + batches
    chunks = {0: [], 1: []}                  # half -> [(t, c_local)]
    base_th = np.zeros((NT, 2), np.int64)
    off = {0: 0, 1: 0}
    for t in range(NT):
        for h in (0, 1):
            base_th[t, h] = off[h]
            for c in range(int(Kth[t, h])):
                chunks[h].append((t, c))
            off[h] += int(Kth[t, h])
    CL, CH = len(chunks[0]), len(chunks[1])
    NBL, NBH = -(-CL // JJ), -(-CH // JJ)

    batches = []
    for h, nb in ((0, NBL), (1, NBH)):
        for b in range(nb):
            cs = chunks[h][b * JJ:(b + 1) * JJ]
            cs = cs + [None] * (JJ - len(cs))
            batches.append((h, cs))
    # emission order: by tile of first real chunk (within a half this is the
    # stream order, so per-half batch indices still match the host arrays)
    batches.sort(key=lambda hb: min(c[0] for c in hb[1] if c is not None))

    # per-batch work items [(j_chunk, t, r)] and (h,t,c,r) -> (bi, item col)
    bitems = []
    bi_tab = np.full((2, NT, maxK, R), -1, np.int64)
    k_tab = np.full((2, NT, maxK, R), -1, np.int64)
    bwh = {0: 0, 1: 0}
    for h, cs in batches:
        bi = bwh[h]
        bwh[h] += 1
        its = []
        for j, tc in enumerate(cs):
            if tc is None:
                continue
            t, c = tc
            for r in range(R):
                if present[t, h, c, r]:
                    bi_tab[h, t, c, r] = bi
                    k_tab[h, t, c, r] = len(its)
                    its.append((j, t, r))
        bitems.append(its)
    NWI = max(len(its) for its in bitems)

    sched = dict(batches=batches, bitems=bitems, NWI=NWI,
                 NBL=NBL, NBH=NBH, CL=CL, CH=CH)

    # per-core host arrays ---------------------------------------------------
    NHI = N_NODES - SPLIT                    # hi table real rows
    ZLO, ZHI = SPLIT, NHI                    # zero-row indices
    per_core = []
    for m in range(N_CORES):
        glo = np.full((NBL * JJ * P,), ZLO, np.int64)
        ghi = np.full((NBH * JJ * P,), ZHI, np.int64)
        dlo = np.full((NBL, P, NWI), PAD_DST, np.float32)
        dhi = np.full((NBH, P, NWI), PAD_DST, np.float32)
        per_core.append([glo, ghi, dlo, dhi])

    for r in range(R):
        nd = rows[r]
        src = cols[r]
        m = core_of[nd]
        t = tslot_of[nd]
        h = half[r]
        sl = (slot_of[nd] - 0).astype(np.int64)
        # rank within (core, t, r, half) group
        key = ((m.astype(np.int64) * NT + t) * 2 + h)
        sort = np.argsort(key, kind="stable")
        ks = key[sort]
        grp_start = np.r_[0, np.flatnonzero(np.diff(ks)) + 1]
        sizes = np.diff(np.r_[grp_start, len(ks)])
        within = np.arange(len(ks)) - np.repeat(grp_start, sizes)
        inv = np.empty_like(sort)
        inv[sort] = np.arange(len(sort))
        within = within[inv]                  # rank of each edge in its group
        ofs = rstart[m, t, h, r] + within     # local pos in (t,h) stream
        c_local = ofs // P
        lane = ofs % P
        pos = (base_th[t, h] + c_local) * P + lane   # gather stream position
        idx_local = np.where(h == 0, src, src - SPLIT).astype(np.int64)
        bi = bi_tab[h, t, c_local, r]
        kk = k_tab[h, t, c_local, r]
        for mm in range(N_CORES):
            sel = m == mm
            hl = h[sel] == 0
            pc = per_core[mm]
            ps, il, dl = pos[sel], idx_local[sel], sl[sel]
            bb, kx, ln = bi[sel], kk[sel], lane[sel]
            pc[0][ps[hl]] = il[hl]
            pc[1][ps[~hl]] = il[~hl]
            pc[2][bb[hl], ln[hl], kx[hl]] = dl[hl]
            pc[3][bb[~hl], ln[~hl], kx[~hl]] = dl[~hl]

    def wrap_idx(flat, nb):
        # [nb*JJ*P] -> [nb, 128, JJ*8] int16 (idx i -> part i%16, col i//16,
        # replicated 8x across partition groups of 16)
        a = flat.reshape(nb, JJ * P // 16, 16).transpose(0, 2, 1)  # [nb,16,S]
        return np.broadcast_to(a[:, None, :, :], (nb, 8, 16, JJ * P // 16)
                               ).reshape(nb, 128, JJ * P // 16).astype(np.int16)

    import ml_dtypes
    arrays = []
    for m in range(N_CORES):
        glo, ghi, dlo, dhi = per_core[m]
        arrays.append(dict(
            idxlo=wrap_idx(glo, NBL), idxhi=wrap_idx(ghi, NBH),
            dstlo=np.ascontiguousarray(dlo).astype(ml_dtypes.bfloat16),
            dsthi=np.ascontiguousarray(dhi).astype(ml_dtypes.bfloat16)))

    # per-core inverse-degree table [128, NT*R]: node (t, sl) rel r at
    # [sl, t*R + r]
    invd = np.zeros((N_CORES, P, NT * R), np.float32)
    for r in range(R):
        invd[core_of, slot_of, tslot_of * R + r] = inv_node[:, r]
    for m in range(N_CORES):
        arrays[m]["invd"] = invd[m]

    # output unshard indices: node -> global out row (core*NT*128 + t*128 + sl)
    out_row = core_of.astype(np.int64) * (NT * P) + tslot_of.astype(np.int64) * P + slot_of
    sched["out_row"] = out_row
    sched["arrays"] = arrays
    sched["prep_s"] = time.time() - t0
    return sched


# ------------------------------------------------------------- device build
def _build_program(sched):
    from concourse import bass, bacc, mybir, tile
    from concourse.masks import make_identity

    f32 = mybir.dt.float32
    b16 = mybir.dt.bfloat16
    f8 = mybir.dt.float8e4
    i16 = mybir.dt.int16
    Alu = mybir.AluOpType
    Act = mybir.ActivationFunctionType

    NBL, NBH = sched["NBL"], sched["NBH"]
    batches = sched["batches"]
    bitems = sched["bitems"]
    NWI = sched["NWI"]
    NHI = N_NODES - SPLIT

    nc = bacc.Bacc("TRN2", target_bir_lowering=False, debug=False,
                   num_devices=N_CORES, num_swdge_queues=4)

    xlo = nc.dram_tensor("xlo", [SPLIT + 1, 2 * DIN], b16, kind="ExternalInput")
    xhi = nc.dram_tensor("xhi", [NHI + 1, 2 * DIN], b16, kind="ExternalInput")
    idxlo = nc.dram_tensor("idxlo", [NBL, P, JJ * 8], i16, kind="ExternalInput")
    idxhi = nc.dram_tensor("idxhi", [NBH, P, JJ * 8], i16, kind="ExternalInput")
    dstlo = nc.dram_tensor("dstlo", [NBL, P, NWI], b16, kind="ExternalInput")
    dsthi = nc.dram_tensor("dsthi", [NBH, P, NWI], b16, kind="ExternalInput")
    invd = nc.dram_tensor("invd", [P, NT * R_REL], f32, kind="ExternalInput")
    wrel = nc.dram_tensor("wrel", [R_REL, B_BASES * 2], f32, kind="ExternalInput")
    wbas = nc.dram_tensor("wbas", [2, B_BASES, DIN, DOUT], f32, kind="ExternalInput")
    bias = nc.dram_tensor("bias", [1, 2 * DOUT], f32, kind="ExternalInput")
    outa = nc.dram_tensor("outa", [NT * P, DOUT], f32, kind="ExternalOutput")
    dbg = None
    if os.environ.get("KERNEL_DEBUG_TAPS"):
        dbg = dict(
            w=nc.dram_tensor("dbg_w", [P, 8 * DOUT], f32, kind="ExternalOutput"),
            bias=nc.dram_tensor("dbg_bias", [P, 2 * DOUT], f32, kind="ExternalOutput"),
            tmp=nc.dram_tensor("dbg_tmp", [P, R_REL * DIN], f32, kind="ExternalOutput"),
            z=nc.dram_tensor("dbg_z", [P, 2 * DOUT], f32, kind="ExternalOutput"),
        )
    outb = nc.dram_tensor("outb", [NT * P, DOUT], f32, kind="ExternalOutput")

    xt = {0: xlo, 1: xhi}
    idxt = {0: idxlo, 1: idxhi}
    dstt = {0: dstlo, 1: dsthi}

    with tile.TileContext(nc) as tc:
        with tc.tile_pool(name="const", bufs=1) as cp:
            iota = cp.tile([P, NWI * P], b16)
            nc.gpsimd.iota(iota[:], pattern=[[0, NWI], [1, P]], base=0,
                           channel_multiplier=0,
                           allow_small_or_imprecise_dtypes=True)
            ident = cp.tile([P, P], f32)
            make_identity(nc, ident[:])
            invsb = cp.tile([P, NT * R_REL], f32)
            nc.sync.dma_start(invsb[:], invd[:])
            ones_row = cp.tile([1, P], f32)
            nc.gpsimd.memset(ones_row[:], 1.0)
            zero_col = cp.tile([1, P], f32)
            nc.gpsimd.memset(zero_col[:], 0.0)
            zero_row = cp.tile([1, 512], f32)
            nc.gpsimd.memset(zero_row[:], 0.0)

            # ---- weight prep: w[r*64+i, o] = sum_b wrel[r, b] * wbas[b, i, o]
            wrel_sb = cp.tile([R_REL, B_BASES * 2], f32)
            nc.sync.dma_start(wrel_sb[:], wrel[:])
            bias_sb = cp.tile([1, 2 * DOUT], f32)
            nc.sync.dma_start(bias_sb[:], bias[:])
            # w_bases for both heads, replicated to both partition halves
            wb2 = {}
            for hd in range(2):
                wb2[hd] = cp.tile([P, B_BASES * DIN], f32, tag=f"wb2_{hd}", name=f"wb2_{hd}")
                src = wbas[hd].rearrange("b i o -> i b o")
                nc.sync.dma_start(wb2[hd][0:DIN, :], src)
                nc.sync.dma_start(wb2[hd][DIN:2 * DIN, :], src)
            # repsel[q, k*128 + p] = 1 if q == 2k + p//64
            # repsel[q, x] = 1 iff x // 64 == q  (x = k*128 + p)
            repsel = cp.tile([R_REL, 4 * P], f32)
            nc.gpsimd.memset(repsel[:], 0.0)
            nc.gpsimd.affine_select(
                out=repsel[:], in_=repsel[:],
                compare_op=mybir.AluOpType.not_equal, fill=1.0,
                base=0, pattern=[[1, R_REL], [0, 64]], channel_multiplier=-1)
            wsb = {0: cp.tile([P, 4 * DOUT], f32, tag="wa", name="wa"),
                   1: cp.tile([P, 4 * DOUT], f32, tag="wb", name="wb")}
            bias_bc = cp.tile([P, 2 * DOUT], f32)

            with tc.tile_pool(name="prep_ps", bufs=1, space="PSUM") as pp, \
                 tc.tile_pool(name="prep_sb", bufs=1) as psb:
                psb_bias = pp.tile([P, 2 * DOUT], f32, tag="pbias")
                nc.tensor.matmul(psb_bias[:], ones_row[:], bias_sb[:],
                                 start=True, stop=True)
                nc.vector.tensor_copy(bias_bc[:], psb_bias[:])
                ps_wrk = pp.tile([P, 4 * B_BASES * 2], f32, tag="pwrk")
                for k in range(4):
                    nc.tensor.matmul(
                        ps_wrk[:, k * 2 * B_BASES:(k + 1) * 2 * B_BASES],
                        repsel[:, k * P:(k + 1) * P], wrel_sb[:],
                        start=True, stop=True)
                wrk = psb.tile([P, 4 * B_BASES * 2], f32, tag="wrk")
                nc.vector.tensor_copy(wrk[:], ps_wrk[:])
                # products + tree add per head per k-chunk
                for hd in range(2):
                    for k in range(4):
                        prod = psb.tile([P, B_BASES * DOUT], f32, tag="prod")
                        for b in range(B_BASES):
                            nc.vector.tensor_scalar(
                                prod[:, b * DOUT:(b + 1) * DOUT],
                                wb2[hd][:, b * DOUT:(b + 1) * DOUT],
                                wrk[:, k * 2 * B_BASES + hd * B_BASES + b:
                                    k * 2 * B_BASES + hd * B_BASES + b + 1],
                                None, Alu.mult)
                        t1 = psb.tile([P, DOUT], f32, tag="t1")
                        t2 = psb.tile([P, DOUT], f32, tag="t2")
                        nc.vector.tensor_add(t1[:], prod[:, 0:DOUT],
                                             prod[:, DOUT:2 * DOUT])
                        nc.vector.tensor_add(t2[:], prod[:, 2 * DOUT:3 * DOUT],
                                             prod[:, 3 * DOUT:4 * DOUT])
                        nc.vector.tensor_add(
                            wsb[hd][:, k * DOUT:(k + 1) * DOUT], t1[:], t2[:])

            if dbg is not None:
                nc.sync.dma_start(dbg["w"][:, 0:4 * DOUT], wsb[0][:])
                nc.sync.dma_start(dbg["w"][:, 4 * DOUT:], wsb[1][:])
                nc.sync.dma_start(dbg["bias"][:], bias_bc[:])

            # ---- main loop
            with tc.tile_pool(name="io", bufs=10) as iop, \
                 tc.tile_pool(name="oh", bufs=5) as ohp, \
                 tc.tile_pool(name="ep", bufs=6) as epp, \
                 tc.tile_pool(name="ps", bufs=2, space="PSUM") as psp:

                feat_ps = {}
                remaining = {}
                for t in range(NT):
                    remaining[t] = sum(
                        1 for its in bitems for (j, tt, r) in its if tt == t)
                bcount = {0: 0, 1: 0}

                def epilogue1(t):
                    # inline at tile close: normalize out of PSUM (ACT engine
                    # only) so the bank frees fast; no PE ops here.
                    fps = feat_ps.pop(t)
                    tmp = epp.tile([P, R_REL * DIN], f32, tag="tmp",
                                   name=f"tmp{t}")
                    for r in range(R_REL):
                        nc.scalar.mul(tmp[:, r * DIN:(r + 1) * DIN],
                                      fps[:, r * DIN:(r + 1) * DIN],
                                      invsb[:, t * R_REL + r:t * R_REL + r + 1])
                    if dbg is not None and t == 0:
                        nc.sync.dma_start(dbg["tmp"][:], tmp[:])
                    return tmp

                def epilogue2(t, tmp):
                    # deferred a few batches: PE transposes + head matmuls run
                    # with inputs long ready, no PE pipeline bubble.
                    psT = psp.tile([P, 512], f32, tag="psT")
                    tmpT = epp.tile([P, 512], f32, tag="tmpT")
                    for k in range(4):
                        nc.tensor.transpose(psT[:, k * P:(k + 1) * P],
                                            tmp[:, k * P:(k + 1) * P], ident[:])
                        if k % 2 == 0:
                            nc.vector.tensor_copy(tmpT[:, k * P:(k + 1) * P],
                                                  psT[:, k * P:(k + 1) * P])
                        else:
                            nc.scalar.copy(tmpT[:, k * P:(k + 1) * P],
                                           psT[:, k * P:(k + 1) * P])
                    zps = psp.tile([P, 2 * DOUT], f32, tag="zps")
                    nc.tensor.matmul(zps[:], zero_col[:],
                                     zero_row[:, 0:2 * DOUT],
                                     start=True, stop=False)
                    for k in range(4):
                        for hd in range(2):
                            nc.tensor.matmul(
                                zps[:, hd * DOUT:(hd + 1) * DOUT],
                                tmpT[:, k * P:(k + 1) * P],
                                wsb[hd][:, k * DOUT:(k + 1) * DOUT],
                                start=False, stop=False)
                    nc.tensor.matmul(zps[:], zero_col[:],
                                     zero_row[:, 0:2 * DOUT],
                                     start=False, stop=True)
                    if dbg is not None and t == 0:
                        zsb = epp.tile([P, 2 * DOUT], f32, tag="zsb")
                        nc.vector.tensor_copy(zsb[:], zps[:])
                        nc.sync.dma_start(dbg["z"][:], zsb[:])
                    ab = epp.tile([P, 2 * DOUT], f32, tag="ab")
                    for hd in range(2):
                        s = slice(hd * DOUT, (hd + 1) * DOUT)
                        nc.vector.scalar_tensor_tensor(
                            ab[:, s], zps[:, s], 0.0, bias_bc[:, s],
                            Alu.max, Alu.add)
                        nc.scalar.activation(ab[:, s], ab[:, s], Act.Exp)
                        nc.scalar.activation(ab[:, s], ab[:, s], Act.Ln,
                                             bias=1.0)
                        nc.vector.tensor_scalar(ab[:, s], ab[:, s], SHIFT,
                                                None, Alu.add)
                    nc.sync.dma_start(outa[t * P:(t + 1) * P, :], ab[:, 0:DOUT])
                    nc.sync.dma_start(outb[t * P:(t + 1) * P, :],
                                      ab[:, DOUT:2 * DOUT])

                max_b = int(os.environ.get("KERNEL_MAX_BATCHES", "0"))
                if max_b:
                    batches = batches[:max_b]
                qrr = 0
                pending_epi = []
                bidx = 0
                def open_tile(t):
                    # bank-open: one full-bank zeroing matmul sets
                    # has_written everywhere; items then purely
                    # accumulate (robust to any PE ordering)
                    feat_ps[t] = psp.tile([P, R_REL * DIN], f32,
                                          tag="feat", name=f"feat{t}")
                    nc.tensor.matmul(feat_ps[t][:], zero_col[:],
                                     zero_row[:], start=True, stop=False)

                def close_tile(t):
                    # bank-close: accumulate zeros over the full bank
                    # (data unchanged) to end the group everywhere
                    nc.tensor.matmul(feat_ps[t][:], zero_col[:],
                                     zero_row[:], start=False, stop=True)
                    pending_epi.append((bidx + 3, t, epilogue1(t)))

                for (h, cs), its in zip(batches, bitems):
                    while pending_epi and pending_epi[0][0] <= bidx:
                        _, pt, ptmp = pending_epi.pop(0)
                        epilogue2(pt, ptmp)
                    bidx += 1
                    bi = bcount[h]
                    bcount[h] += 1
                    nwi = len(its)
                    idx = iop.tile([P, JJ * 8], i16, tag="idx")
                    nc.sync.dma_start(idx[:], idxt[h][bi])
                    dst = iop.tile([P, NWI], b16, tag="dst")
                    nc.sync.dma_start(dst[:], dstt[h][bi])
                    G = iop.tile([P, JJ * 2 * DIN], b16, tag="G")
                    JS = JJ // GSPLIT
                    for s in range(GSPLIT):
                        nc.gpsimd.dma_gather(
                            out_ap=G[:, s * JS * 2 * DIN:(s + 1) * JS * 2 * DIN
                                     ].rearrange("p (c e) -> p c e", e=2 * DIN),
                            in_ap=xt[h][:],
                            idxs_ap=idx[:, s * JS * 8:(s + 1) * JS * 8],
                            num_idxs=JS * P,
                            num_idxs_reg=JS * P,
                            elem_size=2 * DIN,
                            single_packet=False,
                            queue_num=qrr % 4)
                        qrr += 1
                    # extra bump rotates which queue PAIR consecutive batches
                    # co-start on ({0,1},{3,0},{2,3},{1,2},...) instead of
                    # pinning pairs {0,1}/{2,3}
                    qrr += 1
                    oh = ohp.tile([P, NWI * P], b16, tag="oh")
                    nc.vector.tensor_tensor(
                        out=oh[:, 0:nwi * P].rearrange("p (j q) -> p j q", q=P),
                        in0=iota[:, 0:nwi * P].rearrange("p (j q) -> p j q", q=P),
                        in1=dst[:, 0:nwi].unsqueeze(2).to_broadcast(
                            [P, nwi, P]),
                        op=Alu.is_equal)
                    for k, (j, t, r) in enumerate(its):
                        if t not in feat_ps:
                            open_tile(t)
                        nc.tensor.matmul(
                            feat_ps[t][:, r * DIN:(r + 1) * DIN],
                            oh[:, k * P:(k + 1) * P],
                            G[:, j * 2 * DIN:j * 2 * DIN + DIN],
                            start=False, stop=False)
                        remaining[t] -= 1
                        if remaining[t] == 0:
                            close_tile(t)

                # tiles with no edges at all still need their (bias-only)
                # output rows
                ever = {t for its in bitems for (j, t, r) in its}
                for t in range(NT):
                    if t not in ever:
                        open_tile(t)
                        close_tile(t)

                for _, pt, ptmp in pending_epi:
                    epilogue2(pt, ptmp)

    nc.compile()
    return nc


# ------------------------------------------------------------------ kernel
def kernel(X, rows, cols, w_bases_alpha, w_rel_alpha, w_bases_beta,
           w_rel_beta, bias_alpha, bias_beta):
    from concourse.bass_utils import run_bass_kernel_spmd

    X = np.nan_to_num(np.asarray(X, np.float32))
    rows = np.asarray(rows)
    cols = np.asarray(cols)

    sched = _build_schedule(rows.astype(np.int64), cols.astype(np.int64))

    key = (sched["NBL"], sched["NBH"], sched["NWI"])
    if key not in _cache:
        t0 = time.time()
        _cache[key] = _build_program(sched)
        if os.environ.get("KERNEL_VERBOSE"):
            print(f"[kernel] prep {sched['prep_s']:.1f}s, "
                  f"compile {time.time() - t0:.1f}s, "
                  f"chunks lo/hi {sched['CL']}/{sched['CH']}")
    nc = _cache[key]

    import ml_dtypes
    bf16 = ml_dtypes.bfloat16
    NHI = N_NODES - SPLIT
    hi = X.astype(bf16)
    lo = (X - hi.astype(np.float32)).astype(bf16)
    xhl = np.concatenate([hi, lo], axis=1)          # [N, 128] bf16
    xlo = np.zeros((SPLIT + 1, 2 * DIN), bf16)
    xlo[:SPLIT] = xhl[:SPLIT]
    xhi = np.zeros((NHI + 1, 2 * DIN), bf16)
    xhi[:NHI] = xhl[SPLIT:]
    wrel = np.concatenate([np.asarray(w_rel_alpha, np.float32),
                           np.asarray(w_rel_beta, np.float32)], axis=1)
    wbas = np.stack([np.asarray(w_bases_alpha, np.float32),
                     np.asarray(w_bases_beta, np.float32)])
    biases = np.concatenate([np.asarray(bias_alpha, np.float32),
                             np.asarray(bias_beta, np.float32)])[None, :]

    in_maps = []
    for m in range(N_CORES):
        a = sched["arrays"][m]
        in_maps.append(dict(
            xlo=xlo, xhi=xhi,
            idxlo=a["idxlo"], idxhi=a["idxhi"],
            dstlo=a["dstlo"], dsthi=a["dsthi"],
            invd=a["invd"],
            wrel=wrel, wbas=wbas, bias=biases))

    trace = os.environ.get("KERNEL_TRACE", "") not in ("", "0")
    res = run_bass_kernel_spmd(nc, in_maps, core_ids=list(range(N_CORES)),
                               trace=trace)
    if trace and os.environ.get("KERNEL_VERBOSE"):
        print(f"[kernel] HW exec_time_ns: {res.exec_time_ns}")
    kernel.last_exec_time_ns = res.exec_time_ns

    kernel.last_results = res.results
    kernel.last_sched = sched
    out_row = sched["out_row"]
    alla = np.concatenate([res.results[m]["outa"] for m in range(N_CORES)])
    allb = np.concatenate([res.results[m]["outb"] for m in range(N_CORES)])
    alpha = np.ascontiguousarray(alla[out_row])
    beta = np.ascontiguousarray(allb[out_row])
    return alpha, beta


kernel.last_exec_time_ns = None



# revision 67
# speedup vs baseline: 1.0202x; 1.0202x over previous
"""Trainium2 Bass kernel for nn_BetaMPERLGraphConvLayer (relational GNN layer).

Computation (see the problem's reference):
  per relation r: mean-aggregate neighbor features over edges
  (segment-sum by destination + degree normalize), concat the R supports,
  two basis-decomposed linear heads, relu+bias, 1.01+softplus.

Strategy:
  - Destination nodes are packed into 128-node tiles and the tiles are dealt
    across the 8 NeuronCores (host-side balanced packing -> one SPMD
    program).
  - Mixed-relation chunk packing: per (tile, half) a core's edges are sorted
    by relation and chunked every 128 with no per-relation alignment; every
    (chunk, relation) pair present on any core is a work item with its own
    one-hot column (so relation boundaries can straddle chunks).
  - Per 128-edge chunk the kernel gathers the 128 source rows with
    dma_gather (int16 indices -> X split into two <=32768-row half tables),
    builds one-hot [edge, dest-slot] columns on the vector engine
    (iota == dest), and scatter-adds via TensorE:
    psum[dest, feat] += onehot.T @ G_hi (bf16-hi only; rel err ~2e-3).
  - SWDGE descriptor generation is the hard bottleneck; it is spread over
    4 SWDGE queues (num_swdge_queues=4, round-robin queue_num) which
    parallelizes the Q7 desc-gen ucode (~100 desc/us per active queue,
    additive across queues).
  - Inverse degrees 1/(deg+eps) are exact, computed host-side from `rows`
    and shipped as a per-core input table (no degree matmuls on device).
  - Per-tile epilogue is split: normalize out of PSUM inline (ACT engine,
    frees the bank), then 3 batches later the PE transposes + two 512->64
    head matmuls + relu/bias/softplus/+1.01 run with inputs long ready
    (no PE pipeline bubble at tile close).

Measured (8 cores, full problem): 1.269 ms HW exec, rel err ~2e-3
(gate is 2e-2).  History: baseline 3.83 ms (single SWDGE queue, hi+lo
matmuls, device degrees); 1.92 ms after 4 SWDGE queues; 1.82 ms after
host inv-degree + hi-only matmul; 1.75 ms after mixed-relation packing
(466k -> 410k gather descriptors/core); 1.43 ms after JJ=16 -> 8;
1.287 ms after GSPLIT=2 (each batch's gather split into two 512-idx
dma_gather calls on different queues: 512-idx ucode quanta keep all 4
SWDGE queues fed -> sustained ~344 desc/us vs ~250 before); 1.269 ms
after rotating which queue pair consecutive batches co-start on
(extra qrr bump per batch) instead of pinning pairs {0,1}/{2,3}.
Known walls: desc-gen sustains ~344/430 desc/us (per-queue ~100/us,
additive); stream head ~36us + tail ~45us; DVE one-hot ~0.65 ms pure.
Measured SLOWER: JJ=32/16/6/4 variants, GSPLIT=4 (256-idx quanta too
fine, 1.69ms), JJ=16+GSPLIT=4 (same quanta, bigger batches, 1.47ms),
io bufs 12/14, 64KB dma scratch, (lo,hi)-total packing objective,
fp8e4 one-hot (correct but 1.31ms); single_packet=True crashes.
"""

import os
import sys
import time

for _p in ("/opt/trn_rl_repo", "/root/.axon_site/_ro/trn_rl_repo"):
    if os.path.isdir(_p) and _p not in sys.path:
        sys.path.insert(0, _p)

import numpy as np

# ---------------------------------------------------------------- constants
N_NODES = 50000
DIN = 64
DOUT = 64
R_REL = 8
B_BASES = 4
N_CORES = 8
P = 128
EPS = 1e-8
SHIFT = 1.01

SPLIT = 32767          # lo table: rows [0, 32767) + zero row at 32767
NT = 50                # dest tiles per core (50*128*8 = 51200 slots >= 50000)
JJ = 8                 # 128-edge chunks per gather batch
GSPLIT = 2             # gather instructions per batch (round-robin queues)
PAD_DST = 255.0        # one-hot target that never matches iota 0..127

_cache = {}


# ---------------------------------------------------------------- host prep
def _build_schedule(rows, cols):
    """Assign nodes to (core, tile, slot); build per-core edge chunk grids and
    the shared compile-time chunk schedule."""
    t0 = time.time()
    R, E = rows.shape
    TILES = N_CORES * NT

    half = (cols >= SPLIT).astype(np.int64)            # [R, E]
    # per-node degree split by (relation, half): [N, R*2]
    deg = np.zeros((N_NODES, R * 2), np.int64)
    for r in range(R):
        key = rows[r] * 2 + half[r]
        cnt = np.bincount(key, minlength=N_NODES * 2)
        deg[:, 2 * r] = cnt[0::2]
        deg[:, 2 * r + 1] = cnt[1::2]

    # exact per-(relation, node) inverse degree (device ships this as input)
    inv_node = 1.0 / ((deg[:, 0::2] + deg[:, 1::2]).astype(np.float64) + EPS)
    inv_node = inv_node.astype(np.float32)             # [N, R]

    # greedy vector bin-packing: nodes (desc by max group count) -> tiles
    order = np.argsort(-deg.max(1), kind="stable")
    counts = np.zeros((TILES, R * 2), np.int64)
    fill = np.zeros(TILES, np.int64)
    tile_of = np.empty(N_NODES, np.int32)
    slot_of = np.empty(N_NODES, np.int32)
    BIG = 1 << 40
    for n in order:
        d = deg[n]
        cand = (counts + d).max(1)
        cand[fill >= P] = BIG
        t = int(np.argmin(cand))
        tile_of[n] = t
        slot_of[n] = fill[t]
        counts[t] += d
        fill[t] += 1

    # deal tiles to cores: sort by total desc, tile i -> (core i%8, slot i//8)
    tord = np.argsort(-counts.sum(1), kind="stable")
    core_of_tile = np.empty(TILES, np.int32)
    slotT_of_tile = np.empty(TILES, np.int32)
    core_of_tile[tord] = np.arange(TILES) % N_CORES
    slotT_of_tile[tord] = np.arange(TILES) // N_CORES

    core_of = core_of_tile[tile_of]          # [N]
    tslot_of = slotT_of_tile[tile_of]        # [N] tile index within core
    # per (core, tslot, r, half) counts
    cnt4 = np.zeros((N_CORES, NT, R, 2), np.int64)
    for r in range(R):
        key = ((core_of[rows[r]] * NT + tslot_of[rows[r]]) * 2 + half[r])
        c = np.bincount(key, minlength=N_CORES * NT * 2)
        cnt4[:, :, r, :] = c.reshape(N_CORES, NT, 2)

    # mixed-relation chunk packing ------------------------------------------
    # Per (tslot, half): a core's edges are laid out sorted by relation and
    # chunked every 128 with no per-relation alignment; chunk count is the max
    # over cores.  Every (chunk, relation) pair present on ANY core becomes a
    # work item with its own one-hot column (relation boundaries straddle
    # chunks, so a chunk can carry 1-3 items).
    tot_th = cnt4.sum(2)                     # [m, NT, 2]
    Kth = (-(-tot_th // P)).max(0)           # [NT, 2] ceil-div, max over cores
    maxK = int(Kth.max())

    # rstart[m, t, h, r]: offset of relation r inside core m's (t,h) stream
    rstart = np.zeros((N_CORES, NT, 2, R + 1), np.int64)
    rstart[:, :, :, 1:] = np.cumsum(cnt4.transpose(0, 1, 3, 2), axis=3)

    present = np.zeros((NT, 2, maxK, R), bool)
    for m in range(N_CORES):
        for h in (0, 1):
            s = rstart[m, :, h, :-1]
            e = rstart[m, :, h, 1:]
            for t in range(NT):
                for r in range(R):
                    if e[t, r] > s[t, r]:
                        present[t, h, s[t, r] // P:(e[t, r] - 1) // P + 1,
                                r] = True

    # chunk streams + batches
    chunks = {0: [], 1: []}                  # half -> [(t, c_local)]
    base_th = np.zeros((NT, 2), np.int64)
    off = {0: 0, 1: 0}
    for t in range(NT):
        for h in (0, 1):
            base_th[t, h] = off[h]
            for c in range(int(Kth[t, h])):
                chunks[h].append((t, c))
            off[h] += int(Kth[t, h])
    CL, CH = len(chunks[0]), len(chunks[1])
    NBL, NBH = -(-CL // JJ), -(-CH // JJ)

    batches = []
    for h, nb in ((0, NBL), (1, NBH)):
        for b in range(nb):
            cs = chunks[h][b * JJ:(b + 1) * JJ]
            cs = cs + [None] * (JJ - len(cs))
            batches.append((h, cs))
    # emission order: by tile of first real chunk (within a half this is the
    # stream order, so per-half batch indices still match the host arrays)
    batches.sort(key=lambda hb: min(c[0] for c in hb[1] if c is not None))

    # per-batch work items [(j_chunk, t, r)] and (h,t,c,r) -> (bi, item col)
    bitems = []
    bi_tab = np.full((2, NT, maxK, R), -1, np.int64)
    k_tab = np.full((2, NT, maxK, R), -1, np.int64)
    bwh = {0: 0, 1: 0}
    for h, cs in batches:
        bi = bwh[h]
        bwh[h] += 1
        its = []
        for j, tc in enumerate(cs):
            if tc is None:
                continue
            t, c = tc
            for r in range(R):
                if present[t, h, c, r]:
                    bi_tab[h, t, c, r] = bi
                    k_tab[h, t, c, r] = len(its)
                    its.append((j, t, r))
        bitems.append(its)
    NWI = max(len(its) for its in bitems)

    sched = dict(batches=batches, bitems=bitems, NWI=NWI,
                 NBL=NBL, NBH=NBH, CL=CL, CH=CH)

    # per-core host arrays ---------------------------------------------------
    NHI = N_NODES - SPLIT                    # hi table real rows
    ZLO, ZHI = SPLIT, NHI                    # zero-row indices
    per_core = []
    for m in range(N_CORES):
        glo = np.full((NBL * JJ * P,), ZLO, np.int64)
        ghi = np.full((NBH * JJ * P,), ZHI, np.int64)
        dlo = np.full((NBL, P, NWI), PAD_DST, np.float32)
        dhi = np.full((NBH, P, NWI), PAD_DST, np.float32)
        per_core.append([glo, ghi, dlo, dhi])

    for r in range(R):
        nd = rows[r]
        src = cols[r]
        m = core_of[nd]
        t = tslot_of[nd]
        h = half[r]
        sl = (slot_of[nd] - 0).astype(np.int64)
        # rank within (core, t, r, half) group
        key = ((m.astype(np.int64) * NT + t) * 2 + h)
        sort = np.argsort(key, kind="stable")
        ks = key[sort]
        grp_start = np.r_[0, np.flatnonzero(np.diff(ks)) + 1]
        sizes = np.diff(np.r_[grp_start, len(ks)])
        within = np.arange(len(ks)) - np.repeat(grp_start, sizes)
        inv = np.empty_like(sort)
        inv[sort] = np.arange(len(sort))
        within = within[inv]                  # rank of each edge in its group
        ofs = rstart[m, t, h, r] + within     # local pos in (t,h) stream
        c_local = ofs // P
        lane = ofs % P
        pos = (base_th[t, h] + c_local) * P + lane   # gather stream position
        idx_local = np.where(h == 0, src, src - SPLIT).astype(np.int64)
        bi = bi_tab[h, t, c_local, r]
        kk = k_tab[h, t, c_local, r]
        for mm in range(N_CORES):
            sel = m == mm
            hl = h[sel] == 0
            pc = per_core[mm]
            ps, il, dl = pos[sel], idx_local[sel], sl[sel]
            bb, kx, ln = bi[sel], kk[sel], lane[sel]
            pc[0][ps[hl]] = il[hl]
            pc[1][ps[~hl]] = il[~hl]
            pc[2][bb[hl], ln[hl], kx[hl]] = dl[hl]
            pc[3][bb[~hl], ln[~hl], kx[~hl]] = dl[~hl]

    def wrap_idx(flat, nb):
        # [nb*JJ*P] -> [nb, 128, JJ*8] int16 (idx i -> part i%16, col i//16,
        # replicated 8x across partition groups of 16)
        a = flat.reshape(nb, JJ * P // 16, 16).transpose(0, 2, 1)  # [nb,16,S]
        return np.broadcast_to(a[:, None, :, :], (nb, 8, 16, JJ * P // 16)
                               ).reshape(nb, 128, JJ * P // 16).astype(np.int16)

    import ml_dtypes
    arrays = []
    for m in range(N_CORES):
        glo, ghi, dlo, dhi = per_core[m]
        arrays.append(dict(
            idxlo=wrap_idx(glo, NBL), idxhi=wrap_idx(ghi, NBH),
            dstlo=np.ascontiguousarray(dlo).astype(ml_dtypes.bfloat16),
            dsthi=np.ascontiguousarray(dhi).astype(ml_dtypes.bfloat16)))

    # per-core inverse-degree table [128, NT*R]: node (t, sl) rel r at
    # [sl, t*R + r]
    invd = np.zeros((N_CORES, P, NT * R), np.float32)
    for r in range(R):
        invd[core_of, slot_of, tslot_of * R + r] = inv_node[:, r]
    for m in range(N_CORES):
        arrays[m]["invd"] = invd[m]

    # output unshard indices: node -> global out row (core*NT*128 + t*128 + sl)
    out_row = core_of.astype(np.int64) * (NT * P) + tslot_of.astype(np.int64) * P + slot_of
    sched["out_row"] = out_row
    sched["arrays"] = arrays
    sched["prep_s"] = time.time() - t0
    return sched


# ------------------------------------------------------------- device build
def _build_program(sched):
    from concourse import bass, bacc, mybir, tile
    from concourse.masks import make_identity

    f32 = mybir.dt.float32
    b16 = mybir.dt.bfloat16
    f8 = mybir.dt.float8e4
    i16 = mybir.dt.int16
    Alu = mybir.AluOpType
    Act = mybir.ActivationFunctionType

    NBL, NBH = sched["NBL"], sched["NBH"]
    batches = sched["batches"]
    bitems = sched["bitems"]
    NWI = sched["NWI"]
    NHI = N_NODES - SPLIT

    nc = bacc.Bacc("TRN2", target_bir_lowering=False, debug=False,
                   num_devices=N_CORES, num_swdge_queues=4)

    xlo = nc.dram_tensor("xlo", [SPLIT + 1, 2 * DIN], b16, kind="ExternalInput")
    xhi = nc.dram_tensor("xhi", [NHI + 1, 2 * DIN], b16, kind="ExternalInput")
    idxlo = nc.dram_tensor("idxlo", [NBL, P, JJ * 8], i16, kind="ExternalInput")
    idxhi = nc.dram_tensor("idxhi", [NBH, P, JJ * 8], i16, kind="ExternalInput")
    dstlo = nc.dram_tensor("dstlo", [NBL, P, NWI], b16, kind="ExternalInput")
    dsthi = nc.dram_tensor("dsthi", [NBH, P, NWI], b16, kind="ExternalInput")
    invd = nc.dram_tensor("invd", [P, NT * R_REL], f32, kind="ExternalInput")
    wrel = nc.dram_tensor("wrel", [R_REL, B_BASES * 2], f32, kind="ExternalInput")
    wbas = nc.dram_tensor("wbas", [2, B_BASES, DIN, DOUT], f32, kind="ExternalInput")
    bias = nc.dram_tensor("bias", [1, 2 * DOUT], f32, kind="ExternalInput")
    outa = nc.dram_tensor("outa", [NT * P, DOUT], f32, kind="ExternalOutput")
    dbg = None
    if os.environ.get("KERNEL_DEBUG_TAPS"):
        dbg = dict(
            w=nc.dram_tensor("dbg_w", [P, 8 * DOUT], f32, kind="ExternalOutput"),
            bias=nc.dram_tensor("dbg_bias", [P, 2 * DOUT], f32, kind="ExternalOutput"),
            tmp=nc.dram_tensor("dbg_tmp", [P, R_REL * DIN], f32, kind="ExternalOutput"),
            z=nc.dram_tensor("dbg_z", [P, 2 * DOUT], f32, kind="ExternalOutput"),
        )
    outb = nc.dram_tensor("outb", [NT * P, DOUT], f32, kind="ExternalOutput")

    xt = {0: xlo, 1: xhi}
    idxt = {0: idxlo, 1: idxhi}
    dstt = {0: dstlo, 1: dsthi}

    with tile.TileContext(nc) as tc:
        with tc.tile_pool(name="const", bufs=1) as cp:
            iota = cp.tile([P, NWI * P], b16)
            nc.gpsimd.iota(iota[:], pattern=[[0, NWI], [1, P]], base=0,
                           channel_multiplier=0,
                           allow_small_or_imprecise_dtypes=True)
            ident = cp.tile([P, P], f32)
            make_identity(nc, ident[:])
            invsb = cp.tile([P, NT * R_REL], f32)
            nc.sync.dma_start(invsb[:], invd[:])
            ones_row = cp.tile([1, P], f32)
            nc.gpsimd.memset(ones_row[:], 1.0)
            zero_col = cp.tile([1, P], f32)
            nc.gpsimd.memset(zero_col[:], 0.0)
            zero_row = cp.tile([1, 512], f32)
            nc.gpsimd.memset(zero_row[:], 0.0)

            # ---- weight prep: w[r*64+i, o] = sum_b wrel[r, b] * wbas[b, i, o]
            wrel_sb = cp.tile([R_REL, B_BASES * 2], f32)
            nc.sync.dma_start(wrel_sb[:], wrel[:])
            bias_sb = cp.tile([1, 2 * DOUT], f32)
            nc.sync.dma_start(bias_sb[:], bias[:])
            # w_bases for both heads, replicated to both partition halves
            wb2 = {}
            for hd in range(2):
                wb2[hd] = cp.tile([P, B_BASES * DIN], f32, tag=f"wb2_{hd}", name=f"wb2_{hd}")
                src = wbas[hd].rearrange("b i o -> i b o")
                nc.sync.dma_start(wb2[hd][0:DIN, :], src)
                nc.sync.dma_start(wb2[hd][DIN:2 * DIN, :], src)
            # repsel[q, k*128 + p] = 1 if q == 2k + p//64
            # repsel[q, x] = 1 iff x // 64 == q  (x = k*128 + p)
            repsel = cp.tile([R_REL, 4 * P], f32)
            nc.gpsimd.memset(repsel[:], 0.0)
            nc.gpsimd.affine_select(
                out=repsel[:], in_=repsel[:],
                compare_op=mybir.AluOpType.not_equal, fill=1.0,
                base=0, pattern=[[1, R_REL], [0, 64]], channel_multiplier=-1)
            wsb = {0: cp.tile([P, 4 * DOUT], f32, tag="wa", name="wa"),
                   1: cp.tile([P, 4 * DOUT], f32, tag="wb", name="wb")}
            bias_bc = cp.tile([P, 2 * DOUT], f32)

            with tc.tile_pool(name="prep_ps", bufs=1, space="PSUM") as pp, \
                 tc.tile_pool(name="prep_sb", bufs=1) as psb:
                psb_bias = pp.tile([P, 2 * DOUT], f32, tag="pbias")
                nc.tensor.matmul(psb_bias[:], ones_row[:], bias_sb[:],
                                 start=True, stop=True)
                nc.vector.tensor_copy(bias_bc[:], psb_bias[:])
                ps_wrk = pp.tile([P, 4 * B_BASES * 2], f32, tag="pwrk")
                for k in range(4):
                    nc.tensor.matmul(
                        ps_wrk[:, k * 2 * B_BASES:(k + 1) * 2 * B_BASES],
                        repsel[:, k * P:(k + 1) * P], wrel_sb[:],
                        start=True, stop=True)
                wrk = psb.tile([P, 4 * B_BASES * 2], f32, tag="wrk")
                nc.vector.tensor_copy(wrk[:], ps_wrk[:])
                # products + tree add per head per k-chunk
                for hd in range(2):
                    for k in range(4):
                        prod = psb.tile([P, B_BASES * DOUT], f32, tag="prod")
                        for b in range(B_BASES):
                            nc.vector.tensor_scalar(
                                prod[:, b * DOUT:(b + 1) * DOUT],
                                wb2[hd][:, b * DOUT:(b + 1) * DOUT],
                                wrk[:, k * 2 * B_BASES + hd * B_BASES + b:
                                    k * 2 * B_BASES + hd * B_BASES + b + 1],
                                None, Alu.mult)
                        t1 = psb.tile([P, DOUT], f32, tag="t1")
                        t2 = psb.tile([P, DOUT], f32, tag="t2")
                        nc.vector.tensor_add(t1[:], prod[:, 0:DOUT],
                                             prod[:, DOUT:2 * DOUT])
                        nc.vector.tensor_add(t2[:], prod[:, 2 * DOUT:3 * DOUT],
                                             prod[:, 3 * DOUT:4 * DOUT])
                        nc.vector.tensor_add(
                            wsb[hd][:, k * DOUT:(k + 1) * DOUT], t1[:], t2[:])

            if dbg is not None:
                nc.sync.dma_start(dbg["w"][:, 0:4 * DOUT], wsb[0][:])
                nc.sync.dma_start(dbg["w"][:, 4 * DOUT:], wsb[1][:])
                nc.sync.dma_start(dbg["bias"][:], bias_bc[:])

            # ---- main loop
            with tc.tile_pool(name="io", bufs=10) as iop, \
                 tc.tile_pool(name="oh", bufs=5) as ohp, \
                 tc.tile_pool(name="ep", bufs=6) as epp, \
                 tc.tile_pool(name="ps", bufs=2, space="PSUM") as psp:

                feat_ps = {}
                remaining = {}
                for t in range(NT):
                    remaining[t] = sum(
                        1 for its in bitems for (j, tt, r) in its if tt == t)
                bcount = {0: 0, 1: 0}

                def epilogue1(t):
                    # inline at tile close: normalize out of PSUM (ACT engine
                    # only) so the bank frees fast; no PE ops here.
                    fps = feat_ps.pop(t)
                    tmp = epp.tile([P, R_REL * DIN], f32, tag="tmp",
                                   name=f"tmp{t}")
                    for r in range(R_REL):
                        nc.scalar.mul(tmp[:, r * DIN:(r + 1) * DIN],
                                      fps[:, r * DIN:(r + 1) * DIN],
                                      invsb[:, t * R_REL + r:t * R_REL + r + 1])
                    if dbg is not None and t == 0:
                        nc.sync.dma_start(dbg["tmp"][:], tmp[:])
                    return tmp

                def epilogue2(t, tmp):
                    # deferred a few batches: PE transposes + head matmuls run
                    # with inputs long ready, no PE pipeline bubble.
                    psT = psp.tile([P, 512], f32, tag="psT")
                    tmpT = epp.tile([P, 512], f32, tag="tmpT")
                    for k in range(4):
                        nc.tensor.transpose(psT[:, k * P:(k + 1) * P],
                                            tmp[:, k * P:(k + 1) * P], ident[:])
                        if k % 2 == 0:
                            nc.vector.tensor_copy(tmpT[:, k * P:(k + 1) * P],
                                                  psT[:, k * P:(k + 1) * P])
                        else:
                            nc.scalar.copy(tmpT[:, k * P:(k + 1) * P],
                                           psT[:, k * P:(k + 1) * P])
                    zps = psp.tile([P, 2 * DOUT], f32, tag="zps")
                    nc.tensor.matmul(zps[:], zero_col[:],
                                     zero_row[:, 0:2 * DOUT],
                                     start=True, stop=False)
                    for k in range(4):
                        for hd in range(2):
                            nc.tensor.matmul(
                                zps[:, hd * DOUT:(hd + 1) * DOUT],
                                tmpT[:, k * P:(k + 1) * P],
                                wsb[hd][:, k * DOUT:(k + 1) * DOUT],
                                start=False, stop=False)
                    nc.tensor.matmul(zps[:], zero_col[:],
                                     zero_row[:, 0:2 * DOUT],
                                     start=False, stop=True)
                    if dbg is not None and t == 0:
                        zsb = epp.tile([P, 2 * DOUT], f32, tag="zsb")
                        nc.vector.tensor_copy(zsb[:], zps[:])
                        nc.sync.dma_start(dbg["z"][:], zsb[:])
                    ab = epp.tile([P, 2 * DOUT], f32, tag="ab")
                    for hd in range(2):
                        s = slice(hd * DOUT, (hd + 1) * DOUT)
                        nc.vector.scalar_tensor_tensor(
                            ab[:, s], zps[:, s], 0.0, bias_bc[:, s],
                            Alu.max, Alu.add)
                        nc.scalar.activation(ab[:, s], ab[:, s], Act.Exp)
                        nc.scalar.activation(ab[:, s], ab[:, s], Act.Ln,
                                             bias=1.0)
                        nc.vector.tensor_scalar(ab[:, s], ab[:, s], SHIFT,
                                                None, Alu.add)
                    nc.sync.dma_start(outa[t * P:(t + 1) * P, :], ab[:, 0:DOUT])
                    nc.sync.dma_start(outb[t * P:(t + 1) * P, :],
                                      ab[:, DOUT:2 * DOUT])

                max_b = int(os.environ.get("KERNEL_MAX_BATCHES", "0"))
                if max_b:
                    batches = batches[:max_b]
                qrr = 0
                pending_epi = []
                bidx = 0
                def open_tile(t):
                    # bank-open: one full-bank zeroing matmul sets
                    # has_written everywhere; items then purely
                    # accumulate (robust to any PE ordering)
                    feat_ps[t] = psp.tile([P, R_REL * DIN], f32,
                                          tag="feat", name=f"feat{t}")
                    nc.tensor.matmul(feat_ps[t][:], zero_col[:],
                                     zero_row[:], start=True, stop=False)

                def close_tile(t):
                    # bank-close: accumulate zeros over the full bank
                    # (data unchanged) to end the group everywhere
                    nc.tensor.matmul(feat_ps[t][:], zero_col[:],
                                     zero_row[:], start=False, stop=True)
                    pending_epi.append((bidx + 3, t, epilogue1(t)))

                for (h, cs), its in zip(batches, bitems):
                    while pending_epi and pending_epi[0][0] <= bidx:
                        _, pt, ptmp = pending_epi.pop(0)
                        epilogue2(pt, ptmp)
                    bidx += 1
                    bi = bcount[h]
                    bcount[h] += 1
                    nwi = len(its)
                    idx = iop.tile([P, JJ * 8], i16, tag="idx")
                    nc.sync.dma_start(idx[:], idxt[h][bi])
                    dst = iop.tile([P, NWI], b16, tag="dst")
                    nc.sync.dma_start(dst[:], dstt[h][bi])
                    G = iop.tile([P, JJ * 2 * DIN], b16, tag="G")
                    JS = JJ // GSPLIT
                    for s in range(GSPLIT):
                        nc.gpsimd.dma_gather(
                            out_ap=G[:, s * JS * 2 * DIN:(s + 1) * JS * 2 * DIN
                                     ].rearrange("p (c e) -> p c e", e=2 * DIN),
                            in_ap=xt[h][:],
                            idxs_ap=idx[:, s * JS * 8:(s + 1) * JS * 8],
                            num_idxs=JS * P,
                            num_idxs_reg=JS * P,
                            elem_size=2 * DIN,
                            single_packet=False,
                            queue_num=qrr % 4)
                        qrr += 1
                    # extra bump rotates which queue PAIR consecutive batches
                    # co-start on ({0,1},{3,0},{2,3},{1,2},...) instead of
                    # pinning pairs {0,1}/{2,3}.  NOTE: both gathers of a
                    # batch on the SAME queue corrupts results (and is
                    # slower) — keep them on distinct queues.
                    qrr += 1
                    oh = ohp.tile([P, NWI * P], b16, tag="oh")
                    nc.vector.tensor_tensor(
                        out=oh[:, 0:nwi * P].rearrange("p (j q) -> p j q", q=P),
                        in0=iota[:, 0:nwi * P].rearrange("p (j q) -> p j q", q=P),
                        in1=dst[:, 0:nwi].unsqueeze(2).to_broadcast(
                            [P, nwi, P]),
                        op=Alu.is_equal)
                    for k, (j, t, r) in enumerate(its):
                        if t not in feat_ps:
                            open_tile(t)
                        nc.tensor.matmul(
                            feat_ps[t][:, r * DIN:(r + 1) * DIN],
                            oh[:, k * P:(k + 1) * P],
                            G[:, j * 2 * DIN:j * 2 * DIN + DIN],
                            start=False, stop=False)
                        remaining[t] -= 1
                        if remaining[t] == 0:
                            close_tile(t)

                # tiles with no edges at all still need their (bias-only)
                # output rows
                ever = {t for its in bitems for (j, t, r) in its}
                for t in range(NT):
                    if t not in ever:
                        open_tile(t)
                        close_tile(t)

                for _, pt, ptmp in pending_epi:
                    epilogue2(pt, ptmp)

    nc.compile()
    return nc


# ------------------------------------------------------------------ kernel
def kernel(X, rows, cols, w_bases_alpha, w_rel_alpha, w_bases_beta,
           w_rel_beta, bias_alpha, bias_beta):
    from concourse.bass_utils import run_bass_kernel_spmd

    X = np.nan_to_num(np.asarray(X, np.float32))
    rows = np.asarray(rows)
    cols = np.asarray(cols)

    sched = _build_schedule(rows.astype(np.int64), cols.astype(np.int64))

    key = (sched["NBL"], sched["NBH"], sched["NWI"])
    if key not in _cache:
        t0 = time.time()
        _cache[key] = _build_program(sched)
        if os.environ.get("KERNEL_VERBOSE"):
            print(f"[kernel] prep {sched['prep_s']:.1f}s, "
                  f"compile {time.time() - t0:.1f}s, "
                  f"chunks lo/hi {sched['CL']}/{sched['CH']}")
    nc = _cache[key]

    import ml_dtypes
    bf16 = ml_dtypes.bfloat16
    NHI = N_NODES - SPLIT
    hi = X.astype(bf16)
    lo = (X - hi.astype(np.float32)).astype(bf16)
    xhl = np.concatenate([hi, lo], axis=1)          # [N, 128] bf16
    xlo = np.zeros((SPLIT + 1, 2 * DIN), bf16)
    xlo[:SPLIT] = xhl[:SPLIT]
    xhi = np.zeros((NHI + 1, 2 * DIN), bf16)
    xhi[:NHI] = xhl[SPLIT:]
    wrel = np.concatenate([np.asarray(w_rel_alpha, np.float32),
                           np.asarray(w_rel_beta, np.float32)], axis=1)
    wbas = np.stack([np.asarray(w_bases_alpha, np.float32),
                     np.asarray(w_bases_beta, np.float32)])
    biases = np.concatenate([np.asarray(bias_alpha, np.float32),
                             np.asarray(bias_beta, np.float32)])[None, :]

    in_maps = []
    for m in range(N_CORES):
        a = sched["arrays"][m]
        in_maps.append(dict(
            xlo=xlo, xhi=xhi,
            idxlo=a["idxlo"], idxhi=a["idxhi"],
            dstlo=a["dstlo"], dsthi=a["dsthi"],
            invd=a["invd"],
            wrel=wrel, wbas=wbas, bias=biases))

    trace = os.environ.get("KERNEL_TRACE", "") not in ("", "0")
    res = run_bass_kernel_spmd(nc, in_maps, core_ids=list(range(N_CORES)),
                               trace=trace)
    if trace and os.environ.get("KERNEL_VERBOSE"):
        print(f"[kernel] HW exec_time_ns: {res.exec_time_ns}")
    kernel.last_exec_time_ns = res.exec_time_ns

    kernel.last_results = res.results
    kernel.last_sched = sched
    out_row = sched["out_row"]
    alla = np.concatenate([res.results[m]["outa"] for m in range(N_CORES)])
    allb = np.concatenate([res.results[m]["outb"] for m in range(N_CORES)])
    alpha = np.ascontiguousarray(alla[out_row])
    beta = np.ascontiguousarray(allb[out_row])
    return alpha, beta


kernel.last_exec_time_ns = None

